# revision 2
# baseline (speedup 1.0000x reference)
"""Trainium2 Bass kernel for nn_CNNFromScratch (dense 1-D CNN + MLP head).

Strategy
--------
Pure data parallelism: the batch axis (8192) is split across 8 NeuronCores
(1024 samples each); conv kernels and MLP weights are replicated.

Per core, everything is expressed as TensorE matmuls with the contraction
(input channels x taps) on the partition axis:

  - x is pre-transposed on host to (C=512, B, W=20) and cast to bf16, so a
    c-chunk tile loads as (128 partitions, bt*20) with perfectly contiguous
    per-partition DMA runs.
  - conv_k == sum over taps of  W_tap^T @ x[:, :, w+tap]  accumulated in PSUM.
  - Activations stay on-chip (SBUF, bf16) between layers; layout is
    (C_out partitions, w-major * batch free), which feeds the next conv's
    matmuls with plain contiguous slices.
  - maxpool = DVE tensor_max of two strided slices; MLP = accumulated
    matmuls over (channel, pooled-position) chunks.

Matmul inputs are bf16 (1 cycle/row on PE), accumulation is fp32 in PSUM.
"""

import sys

sys.path.insert(0, "/opt/trn_rl_repo")

import numpy as np
import ml_dtypes

N_CORES = 8
B, E, W = 8192, 512, 20
BC = B // N_CORES  # samples per core
# Batch tiles per core: small first tile shrinks the un-hidden DMA prologue.
TILES = [128, 384, 512]
assert sum(TILES) == BC

BF16 = ml_dtypes.bfloat16

_compiled = {}


def _build():
    import concourse.bass as bass
    from concourse import bacc, mybir
    import concourse.tile as tile

    dt = mybir.dt
    AF = mybir.ActivationFunctionType

    nc = bacc.Bacc(
        "TRN2",
        target_bir_lowering=False,
        debug=False,
        enable_asserts=False,
        num_devices=N_CORES,
    )

    x_d = nc.dram_tensor("x", (E, BC, W), dt.bfloat16, kind="ExternalInput").ap()
    w1_d = nc.dram_tensor("w1", (512, 3 * 64), dt.bfloat16, kind="ExternalInput").ap()
    w2_d = nc.dram_tensor("w2", (64, 5 * 128), dt.bfloat16, kind="ExternalInput").ap()
    w3_d = nc.dram_tensor("w3", (128, 7 * 256), dt.bfloat16, kind="ExternalInput").ap()
    m1_d = nc.dram_tensor("m1", (1024, 256), dt.bfloat16, kind="ExternalInput").ap()
    m2_d = nc.dram_tensor("m2", (256, 128), dt.bfloat16, kind="ExternalInput").ap()
    m3_d = nc.dram_tensor("m3", (128, 1), dt.bfloat16, kind="ExternalInput").ap()
    y_d = nc.dram_tensor("y", (1, BC), dt.float32, kind="ExternalOutput").ap()

    with tile.TileContext(nc) as tc:
        with (
            tc.tile_pool(name="sb", bufs=1) as sb,
            tc.tile_pool(name="ps", bufs=2, space="PSUM") as ps,
        ):
            # ---- weights (resident for the whole kernel) ----
            w1_sb = []
            for q in range(4):
                t = sb.tile([128, 3 * 64], dt.bfloat16, tag=f"w1_{q}")
                nc.sync.dma_start(t[:], w1_d[q * 128 : (q + 1) * 128, :])
                w1_sb.append(t)
            w2_sb = sb.tile([64, 5 * 128], dt.bfloat16, tag="w2")
            nc.sync.dma_start(w2_sb[:], w2_d[:, :])
            w3_sb = sb.tile([128, 7 * 256], dt.bfloat16, tag="w3")
            nc.sync.dma_start(w3_sb[:], w3_d[:, :])
            m1_sb = []
            for wp in range(4):
                row = []
                for q in range(2):
                    t = sb.tile([128, 256], dt.bfloat16, tag=f"m1_{wp}_{q}")
                    r0 = wp * 256 + q * 128
                    nc.sync.dma_start(t[:], m1_d[r0 : r0 + 128, :])
                    row.append(t)
                m1_sb.append(row)
            m2_sb = []
            for q in range(2):
                t = sb.tile([128, 128], dt.bfloat16, tag=f"m2_{q}")
                nc.sync.dma_start(t[:], m2_d[q * 128 : (q + 1) * 128, :])
                m2_sb.append(t)
            m3_sb = sb.tile([128, 1], dt.bfloat16, tag="m3")
            nc.sync.dma_start(m3_sb[:], m3_d[:, :])

            # ---- per-batch-tile pipeline ----
            boff = 0
            for bt in TILES:
                # x: 4 c-chunk tiles, (128, bt, 20); tag-shared slots sized to
                # the largest bt so later tiles prefetch into freed slots.
                x_sb = []
                for q in range(4):
                    t = sb.tile([128, max(TILES) * 20], dt.bfloat16, tag="x", bufs=5)
                    t3 = t[:, : bt * 20].rearrange("p (b w) -> p b w", w=W)
                    nc.sync.dma_start(
                        t3, x_d[q * 128 : (q + 1) * 128, boff : boff + bt, :]
                    )
                    x_sb.append(t3)

                # conv1: (B,512,20) -> relu -> (B,64,18)
                h1 = sb.tile([64, 18 * bt], dt.bfloat16, tag="h1")
                for w in range(18):
                    p1 = ps.tile([64, bt], dt.float32, tag="c1")
                    for q in range(4):
                        for k in range(3):
                            nc.tensor.matmul(
                                p1[:],
                                w1_sb[q][:, k * 64 : (k + 1) * 64],
                                x_sb[q][:, :, w + k],
                                start=(q == 0 and k == 0),
                                stop=(q == 3 and k == 2),
                            )
                    nc.scalar.activation(h1[:, w * bt : (w + 1) * bt], p1[:], AF.Relu)

                # conv2: -> relu -> (B,128,14)
                h2 = sb.tile([128, 14 * bt], dt.bfloat16, tag="h2")
                for w in range(14):
                    p2 = ps.tile([128, bt], dt.float32, tag="c2")
                    for k in range(5):
                        nc.tensor.matmul(
                            p2[:],
                            w2_sb[:, k * 128 : (k + 1) * 128],
                            h1[:, (w + k) * bt : (w + k + 1) * bt],
                            start=(k == 0),
                            stop=(k == 4),
                        )
                    nc.vector.tensor_relu(h2[:, w * bt : (w + 1) * bt], p2[:])

                # conv3: -> relu -> (B,256,8) as two 128-channel tiles
                h3 = [
                    sb.tile([128, 8 * bt], dt.bfloat16, tag=f"h3_{m}", name=f"h3_{m}") for m in range(2)
                ]
                for w in range(8):
                    for m in range(2):
                        p3 = ps.tile([128, bt], dt.float32, tag="c3")
                        for k in range(7):
                            nc.tensor.matmul(
                                p3[:],
                                w3_sb[:, k * 256 + m * 128 : k * 256 + (m + 1) * 128],
                                h2[:, (w + k) * bt : (w + k + 1) * bt],
                                start=(k == 0),
                                stop=(k == 6),
                            )
                        nc.vector.tensor_relu(h3[m][:, w * bt : (w + 1) * bt], p3[:])

                # maxpool k=2 s=2: (B,256,8) -> (B,256,4)
                pooled = [
                    sb.tile([128, 4 * bt], dt.bfloat16, tag=f"pool_{m}", name=f"pool_{m}")
                    for m in range(2)
                ]
                for m in range(2):
                    for p in range(4):
                        nc.vector.tensor_max(
                            pooled[m][:, p * bt : (p + 1) * bt],
                            h3[m][:, (2 * p) * bt : (2 * p + 1) * bt],
                            h3[m][:, (2 * p + 1) * bt : (2 * p + 2) * bt],
                        )

                # mlp1: (B,1024)->(B,256), f = c*4 + wp
                g1 = [sb.tile([128, bt], dt.bfloat16, tag=f"g1_{j}", name=f"g1_{j}") for j in range(2)]
                for j in range(2):
                    pm = ps.tile([128, bt], dt.float32, tag="m")
                    for wp in range(4):
                        for q in range(2):
                            nc.tensor.matmul(
                                pm[:],
                                m1_sb[wp][q][:, j * 128 : (j + 1) * 128],
                                pooled[q][:, wp * bt : (wp + 1) * bt],
                                start=(wp == 0 and q == 0),
                                stop=(wp == 3 and q == 1),
                            )
                    nc.vector.tensor_relu(g1[j][:], pm[:])

                # mlp2: (B,256)->(B,128)
                g2 = sb.tile([128, bt], dt.bfloat16, tag="g2")
                pm = ps.tile([128, bt], dt.float32, tag="m")
                for q in range(2):
                    nc.tensor.matmul(
                        pm[:], m2_sb[q][:], g1[q][:], start=(q == 0), stop=(q == 1)
                    )
                nc.vector.tensor_relu(g2[:], pm[:])

                # mlp3: (B,128)->(B,1)
                pm = ps.tile([1, bt], dt.float32, tag="m")
                nc.tensor.matmul(pm[:], m3_sb[:], g2[:], start=True, stop=True)
                y_sb = sb.tile([1, max(TILES)], dt.float32, tag="y_sb", bufs=2)
                nc.vector.tensor_copy(y_sb[:, :bt], pm[:])
                nc.sync.dma_start(y_d[:, boff : boff + bt], y_sb[:, :bt])

                boff += bt

    nc.compile()
    return nc


def _prep_inputs(x, kernel_1, kernel_2, kernel_3, mlp_weight_1, mlp_weight_2, mlp_weight_3):
    """Host-side sharding + layout prep. Returns in_maps (one dict per core)."""
    w1 = np.ascontiguousarray(
        kernel_1.transpose(1, 2, 0).reshape(512, 3 * 64)
    ).astype(BF16)
    w2 = np.ascontiguousarray(
        kernel_2.transpose(1, 2, 0).reshape(64, 5 * 128)
    ).astype(BF16)
    w3 = np.ascontiguousarray(
        kernel_3.transpose(1, 2, 0).reshape(128, 7 * 256)
    ).astype(BF16)
    # W1 row f = c*4 + wp  ->  m1 row = wp*256 + c
    m1 = np.ascontiguousarray(
        mlp_weight_1.reshape(256, 4, 256).transpose(1, 0, 2).reshape(1024, 256)
    ).astype(BF16)
    m2 = mlp_weight_2.astype(BF16)
    m3 = mlp_weight_3.astype(BF16)

    xb = x.astype(BF16)
    in_maps = []
    for c in range(N_CORES):
        xc = np.ascontiguousarray(
            xb[c * BC : (c + 1) * BC].transpose(1, 0, 2)
        )  # (512, BC, 20)
        in_maps.append(
            {"x": xc, "w1": w1, "w2": w2, "w3": w3, "m1": m1, "m2": m2, "m3": m3}
        )
    return in_maps


def run(inputs, trace=False, **kw):
    """Compile (cached), run on 8 cores, return (y_full, BassKernelResults)."""
    from concourse import bass_utils

    if "nc" not in _compiled:
        _compiled["nc"] = _build()
    nc = _compiled["nc"]
    in_maps = _prep_inputs(**inputs)
    res = bass_utils.run_bass_kernel_spmd(
        nc, in_maps, core_ids=list(range(N_CORES)), trace=trace, **kw
    )
    y = np.concatenate(
        [res.results[c]["y"].reshape(BC, 1) for c in range(N_CORES)], axis=0
    )
    return y.astype(np.float32), res


def kernel(**inputs):
    inputs = {k: np.asarray(v) for k, v in inputs.items()}
    y, _ = run(inputs)
    return y


if __name__ == "__main__":
    rng = np.random.default_rng(0)
    inputs = {
        "x": rng.standard_normal((B, E, W), dtype=np.float32),
        "kernel_1": rng.standard_normal((64, 512, 3), dtype=np.float32),
        "kernel_2": rng.standard_normal((128, 64, 5), dtype=np.float32),
        "kernel_3": rng.standard_normal((256, 128, 7), dtype=np.float32),
        "mlp_weight_1": rng.standard_normal((1024, 256), dtype=np.float32),
        "mlp_weight_2": rng.standard_normal((256, 128), dtype=np.float32),
        "mlp_weight_3": rng.standard_normal((128, 1), dtype=np.float32),
    }
    y = kernel(**inputs)
    print("out", y.shape, y.dtype, y[:4, 0])


# revision 6
# speedup vs baseline: 2.3474x; 2.3474x over previous
"""Trainium2 Bass kernel for nn_CNNFromScratch (dense 1-D CNN + MLP head).

Strategy
--------
Pure data parallelism: the batch axis (8192) is split across 8 NeuronCores
(1024 samples each); conv kernels and MLP weights are replicated.

Per core, everything is expressed as TensorE matmuls with the contraction
(input channels x taps) on the partition axis:

  - x is pre-transposed on host to (C=512, B, W=20) and cast to bf16, so a
    c-chunk tile loads as (128 partitions, bt*20) with perfectly contiguous
    per-partition DMA runs.
  - conv_k == sum over taps of  W_tap^T @ x[:, :, w+tap]  accumulated in PSUM.
  - Activations stay on-chip (SBUF, bf16) between layers; layout is
    (C_out partitions, w-major * batch free), which feeds the next conv's
    matmuls with plain contiguous slices.
  - maxpool = DVE tensor_max of two strided slices; MLP = accumulated
    matmuls over (channel, pooled-position) chunks.

Matmul inputs are bf16 (1 cycle/row on PE), accumulation is fp32 in PSUM.
"""

import sys

sys.path.insert(0, "/opt/trn_rl_repo")

import numpy as np
import ml_dtypes

N_CORES = 8
B, E, W = 8192, 512, 20
BC = B // N_CORES  # samples per core
# Batch tiles per core: small first tile shrinks the un-hidden DMA prologue.
TILES = [128, 384, 512]
assert sum(TILES) == BC

BF16 = ml_dtypes.bfloat16

_compiled = {}


def _build():
    import concourse.bass as bass
    from concourse import bacc, mybir
    import concourse.tile as tile

    dt = mybir.dt
    AF = mybir.ActivationFunctionType

    nc = bacc.Bacc(
        "TRN2",
        target_bir_lowering=False,
        debug=False,
        enable_asserts=False,
        num_devices=N_CORES,
    )

    x_d = nc.dram_tensor("x", (E, W, BC), dt.bfloat16, kind="ExternalInput").ap()
    w1_d = nc.dram_tensor("w1", (512, 3 * 64), dt.bfloat16, kind="ExternalInput").ap()
    w2_d = nc.dram_tensor("w2", (64, 5 * 128), dt.bfloat16, kind="ExternalInput").ap()
    w3_d = nc.dram_tensor("w3", (128, 7 * 256), dt.bfloat16, kind="ExternalInput").ap()
    m1_d = nc.dram_tensor("m1", (1024, 256), dt.bfloat16, kind="ExternalInput").ap()
    m2_d = nc.dram_tensor("m2", (256, 128), dt.bfloat16, kind="ExternalInput").ap()
    m3_d = nc.dram_tensor("m3", (128, 1), dt.bfloat16, kind="ExternalInput").ap()
    y_d = nc.dram_tensor("y", (1, BC), dt.float32, kind="ExternalOutput").ap()

    with tile.TileContext(nc) as tc:
        with (
            tc.tile_pool(name="sb", bufs=1) as sb,
            tc.tile_pool(name="ps", bufs=2, space="PSUM") as ps,
        ):
            # ---- weights (resident for the whole kernel) ----
            w1_sb = []
            for q in range(4):
                t = sb.tile([128, 3 * 64], dt.bfloat16, tag=f"w1_{q}")
                nc.sync.dma_start(t[:], w1_d[q * 128 : (q + 1) * 128, :])
                w1_sb.append(t)
            w2_sb = sb.tile([64, 5 * 128], dt.bfloat16, tag="w2")
            nc.sync.dma_start(w2_sb[:], w2_d[:, :])
            w3_sb = sb.tile([128, 7 * 256], dt.bfloat16, tag="w3")
            nc.sync.dma_start(w3_sb[:], w3_d[:, :])
            m1_sb = []
            for wp in range(4):
                row = []
                for q in range(2):
                    t = sb.tile([128, 256], dt.bfloat16, tag=f"m1_{wp}_{q}")
                    r0 = wp * 256 + q * 128
                    nc.sync.dma_start(t[:], m1_d[r0 : r0 + 128, :])
                    row.append(t)
                m1_sb.append(row)
            m2_sb = []
            for q in range(2):
                t = sb.tile([128, 128], dt.bfloat16, tag=f"m2_{q}")
                nc.sync.dma_start(t[:], m2_d[q * 128 : (q + 1) * 128, :])
                m2_sb.append(t)
            m3_sb = sb.tile([128, 1], dt.bfloat16, tag="m3")
            nc.sync.dma_start(m3_sb[:], m3_d[:, :])

            # ---- per-batch-tile pipeline ----
            boff = 0
            for bt in TILES:
                # x: 4 c-chunk tiles, (128, bt, 20); tag-shared slots sized to
                # the largest bt so later tiles prefetch into freed slots.
                # SBUF layout is w-major (free = w*bt + b) so conv1's matmul
                # rhs slices are contiguous — strided moving operands stream
                # at half rate and keep the PE clock-gate cold.
                x_sb = []
                for q in range(4):
                    t = sb.tile([128, max(TILES) * 20], dt.bfloat16, tag="x", bufs=5)
                    t3 = t[:, : bt * 20].rearrange("p (w b) -> p w b", b=bt)
                    nc.sync.dma_start(
                        t3, x_d[q * 128 : (q + 1) * 128, :, boff : boff + bt]
                    )
                    x_sb.append(t[:, : bt * 20])

                # conv1: (B,512,20) -> relu -> (B,64,18)
                h1 = sb.tile([64, 18 * bt], dt.bfloat16, tag="h1")
                for w in range(18):
                    p1 = ps.tile([64, bt], dt.float32, tag="c1")
                    for q in range(4):
                        for k in range(3):
                            nc.tensor.matmul(
                                p1[:],
                                w1_sb[q][:, k * 64 : (k + 1) * 64],
                                x_sb[q][:, (w + k) * bt : (w + k + 1) * bt],
                                start=(q == 0 and k == 0),
                                stop=(q == 3 and k == 2),
                            )
                    nc.scalar.activation(h1[:, w * bt : (w + 1) * bt], p1[:], AF.Relu)

                # conv2: -> relu -> (B,128,14)
                h2 = sb.tile([128, 14 * bt], dt.bfloat16, tag="h2")
                for w in range(14):
                    p2 = ps.tile([128, bt], dt.float32, tag="c2")
                    for k in range(5):
                        nc.tensor.matmul(
                            p2[:],
                            w2_sb[:, k * 128 : (k + 1) * 128],
                            h1[:, (w + k) * bt : (w + k + 1) * bt],
                            start=(k == 0),
                            stop=(k == 4),
                        )
                    nc.vector.tensor_relu(h2[:, w * bt : (w + 1) * bt], p2[:])

                # conv3: -> relu -> (B,256,8) as two 128-channel tiles
                h3 = [
                    sb.tile([128, 8 * bt], dt.bfloat16, tag=f"h3_{m}", name=f"h3_{m}") for m in range(2)
                ]
                for w in range(8):
                    for m in range(2):
                        p3 = ps.tile([128, bt], dt.float32, tag="c3")
                        for k in range(7):
                            nc.tensor.matmul(
                                p3[:],
                                w3_sb[:, k * 256 + m * 128 : k * 256 + (m + 1) * 128],
                                h2[:, (w + k) * bt : (w + k + 1) * bt],
                                start=(k == 0),
                                stop=(k == 6),
                            )
                        nc.vector.tensor_relu(h3[m][:, w * bt : (w + 1) * bt], p3[:])

                # maxpool k=2 s=2: (B,256,8) -> (B,256,4)
                pooled = [
                    sb.tile([128, 4 * bt], dt.bfloat16, tag=f"pool_{m}", name=f"pool_{m}")
                    for m in range(2)
                ]
                for m in range(2):
                    for p in range(4):
                        nc.vector.tensor_max(
                            pooled[m][:, p * bt : (p + 1) * bt],
                            h3[m][:, (2 * p) * bt : (2 * p + 1) * bt],
                            h3[m][:, (2 * p + 1) * bt : (2 * p + 2) * bt],
                        )

                # mlp1: (B,1024)->(B,256), f = c*4 + wp
                g1 = [sb.tile([128, bt], dt.bfloat16, tag=f"g1_{j}", name=f"g1_{j}") for j in range(2)]
                for j in range(2):
                    pm = ps.tile([128, bt], dt.float32, tag="m")
                    for wp in range(4):
                        for q in range(2):
                            nc.tensor.matmul(
                                pm[:],
                                m1_sb[wp][q][:, j * 128 : (j + 1) * 128],
                                pooled[q][:, wp * bt : (wp + 1) * bt],
                                start=(wp == 0 and q == 0),
                                stop=(wp == 3 and q == 1),
                            )
                    nc.vector.tensor_relu(g1[j][:], pm[:])

                # mlp2: (B,256)->(B,128)
                g2 = sb.tile([128, bt], dt.bfloat16, tag="g2")
                pm = ps.tile([128, bt], dt.float32, tag="m")
                for q in range(2):
                    nc.tensor.matmul(
                        pm[:], m2_sb[q][:], g1[q][:], start=(q == 0), stop=(q == 1)
                    )
                nc.vector.tensor_relu(g2[:], pm[:])

                # mlp3: (B,128)->(B,1)
                pm = ps.tile([1, bt], dt.float32, tag="m")
                nc.tensor.matmul(pm[:], m3_sb[:], g2[:], start=True, stop=True)
                y_sb = sb.tile([1, max(TILES)], dt.float32, tag="y_sb", bufs=2)
                nc.vector.tensor_copy(y_sb[:, :bt], pm[:])
                nc.sync.dma_start(y_d[:, boff : boff + bt], y_sb[:, :bt])

                boff += bt

    nc.compile()
    return nc


def _prep_inputs(x, kernel_1, kernel_2, kernel_3, mlp_weight_1, mlp_weight_2, mlp_weight_3):
    """Host-side sharding + layout prep. Returns in_maps (one dict per core)."""
    w1 = np.ascontiguousarray(
        kernel_1.transpose(1, 2, 0).reshape(512, 3 * 64)
    ).astype(BF16)
    w2 = np.ascontiguousarray(
        kernel_2.transpose(1, 2, 0).reshape(64, 5 * 128)
    ).astype(BF16)
    w3 = np.ascontiguousarray(
        kernel_3.transpose(1, 2, 0).reshape(128, 7 * 256)
    ).astype(BF16)
    # W1 row f = c*4 + wp  ->  m1 row = wp*256 + c
    m1 = np.ascontiguousarray(
        mlp_weight_1.reshape(256, 4, 256).transpose(1, 0, 2).reshape(1024, 256)
    ).astype(BF16)
    m2 = mlp_weight_2.astype(BF16)
    m3 = mlp_weight_3.astype(BF16)

    xb = x.astype(BF16)
    in_maps = []
    for c in range(N_CORES):
        xc = np.ascontiguousarray(
            xb[c * BC : (c + 1) * BC].transpose(1, 2, 0)
        )  # (512, 20, BC)
        in_maps.append(
            {"x": xc, "w1": w1, "w2": w2, "w3": w3, "m1": m1, "m2": m2, "m3": m3}
        )
    return in_maps


def run(inputs, trace=False, **kw):
    """Compile (cached), run on 8 cores, return (y_full, BassKernelResults)."""
    from concourse import bass_utils

    if "nc" not in _compiled:
        _compiled["nc"] = _build()
    nc = _compiled["nc"]
    in_maps = _prep_inputs(**inputs)
    res = bass_utils.run_bass_kernel_spmd(
        nc, in_maps, core_ids=list(range(N_CORES)), trace=trace, **kw
    )
    y = np.concatenate(
        [res.results[c]["y"].reshape(BC, 1) for c in range(N_CORES)], axis=0
    )
    return y.astype(np.float32), res


def kernel(**inputs):
    inputs = {k: np.asarray(v) for k, v in inputs.items()}
    y, _ = run(inputs)
    return y


if __name__ == "__main__":
    rng = np.random.default_rng(0)
    inputs = {
        "x": rng.standard_normal((B, E, W), dtype=np.float32),
        "kernel_1": rng.standard_normal((64, 512, 3), dtype=np.float32),
        "kernel_2": rng.standard_normal((128, 64, 5), dtype=np.float32),
        "kernel_3": rng.standard_normal((256, 128, 7), dtype=np.float32),
        "mlp_weight_1": rng.standard_normal((1024, 256), dtype=np.float32),
        "mlp_weight_2": rng.standard_normal((256, 128), dtype=np.float32),
        "mlp_weight_3": rng.standard_normal((128, 1), dtype=np.float32),
    }
    y = kernel(**inputs)
    print("out", y.shape, y.dtype, y[:4, 0])


# revision 18
# speedup vs baseline: 3.0866x; 1.3149x over previous
"""Trainium2 Bass kernel for nn_CNNFromScratch (dense 1-D CNN + MLP head).

Strategy
--------
Pure data parallelism: the batch axis (8192) is split across 8 NeuronCores
(1024 samples each); conv kernels and MLP weights are replicated.

Per core, everything is expressed as TensorE matmuls with the contraction
(input channels x taps) on the partition axis:

  - x is pre-transposed on host to (C=512, B, W=20) and cast to bf16, so a
    c-chunk tile loads as (128 partitions, bt*20) with perfectly contiguous
    per-partition DMA runs.
  - conv_k == sum over taps of  W_tap^T @ x[:, :, w+tap]  accumulated in PSUM.
  - Activations stay on-chip (SBUF, bf16) between layers; layout is
    (C_out partitions, w-major * batch free), which feeds the next conv's
    matmuls with plain contiguous slices.
  - maxpool = DVE tensor_max of two strided slices; MLP = accumulated
    matmuls over (channel, pooled-position) chunks.

Matmul inputs are bf16 (1 cycle/row on PE), accumulation is fp32 in PSUM.
"""

import sys

sys.path.insert(0, "/opt/trn_rl_repo")

import numpy as np
import ml_dtypes

N_CORES = 8
B, E, W = 8192, 512, 20
BC = B // N_CORES  # samples per core
# Batch tiles per core: small first tile shrinks the un-hidden DMA prologue.
TILES = [128, 384, 512]
assert sum(TILES) == BC

BF16 = ml_dtypes.bfloat16

_compiled = {}


def _build():
    import concourse.bass as bass
    from concourse import bacc, mybir
    import concourse.tile as tile

    dt = mybir.dt
    AF = mybir.ActivationFunctionType

    nc = bacc.Bacc(
        "TRN2",
        target_bir_lowering=False,
        debug=False,
        enable_asserts=False,
        num_devices=N_CORES,
    )

    x_d = nc.dram_tensor("x", (E, W, BC), dt.bfloat16, kind="ExternalInput").ap()
    w1_d = nc.dram_tensor("w1", (512, 3 * 64), dt.bfloat16, kind="ExternalInput").ap()
    w2_d = nc.dram_tensor("w2", (128, 6 * 128), dt.bfloat16, kind="ExternalInput").ap()
    w3_d = nc.dram_tensor("w3", (128, 7 * 256), dt.bfloat16, kind="ExternalInput").ap()
    m1_d = nc.dram_tensor("m1", (1024, 256), dt.bfloat16, kind="ExternalInput").ap()
    m2_d = nc.dram_tensor("m2", (256, 128), dt.bfloat16, kind="ExternalInput").ap()
    m3_d = nc.dram_tensor("m3", (128, 1), dt.bfloat16, kind="ExternalInput").ap()
    y_d = nc.dram_tensor("y", (1, BC), dt.float32, kind="ExternalOutput").ap()

    with tile.TileContext(nc) as tc:
        with (
            tc.tile_pool(name="sb", bufs=1) as sb,
            tc.tile_pool(name="ps", bufs=2, space="PSUM") as ps,
        ):
            # ---- weights (resident for the whole kernel) ----
            # conv1 weights + tile-0 x are emitted first so the PE can start
            # ~8us in; the remaining weights load under tile-0's conv1.
            w1_sb = []
            for q in range(4):
                t = sb.tile([128, 3 * 64], dt.bfloat16, tag=f"w1_{q}")
                nc.sync.dma_start(t[:], w1_d[q * 128 : (q + 1) * 128, :])
                w1_sb.append(t)

            def load_x(bt, boff):
                x_sb = []
                for q in range(4):
                    t = sb.tile(
                        [128, max(TILES) * 20],
                        dt.bfloat16,
                        tag="x",
                        bufs=5,
                        name=f"x_{q}",
                    )
                    t3 = t[:, : bt * 20].rearrange("p (w b) -> p w b", b=bt)
                    nc.sync.dma_start(
                        t3, x_d[q * 128 : (q + 1) * 128, :, boff : boff + bt]
                    )
                    x_sb.append(t[:, : bt * 20])
                return x_sb

            w2_sb = sb.tile([128, 6 * 128], dt.bfloat16, tag="w2")
            nc.sync.dma_start(w2_sb[:], w2_d[:, :])
            w3_sb = sb.tile([128, 7 * 256], dt.bfloat16, tag="w3")
            nc.sync.dma_start(w3_sb[:], w3_d[:, :])
            m1_sb = []
            for wp in range(4):
                row = []
                for q in range(2):
                    t = sb.tile([128, 256], dt.bfloat16, tag=f"m1_{wp}_{q}")
                    r0 = wp * 256 + q * 128
                    nc.sync.dma_start(t[:], m1_d[r0 : r0 + 128, :])
                    row.append(t)
                m1_sb.append(row)
            m2_sb = []
            for q in range(2):
                t = sb.tile([128, 128], dt.bfloat16, tag=f"m2_{q}")
                nc.sync.dma_start(t[:], m2_d[q * 128 : (q + 1) * 128, :])
                m2_sb.append(t)
            m3_sb = sb.tile([128, 1], dt.bfloat16, tag="m3")
            nc.sync.dma_start(m3_sb[:], m3_d[:, :])

            # ---- per-batch-tile pipeline ----
            boff = 0
            for ti, bt in enumerate(TILES):
                # x: 4 c-chunk tiles, (128, 20, bt); tag-shared slots sized to
                # the largest bt so later tiles prefetch into freed slots.
                # SBUF layout is w-major (free = w*bt + b) so conv1's matmul
                # rhs slices are contiguous — strided moving operands stream
                # at half rate and keep the PE clock-gate cold.
                x_sb = load_x(bt, boff)

                # conv1: (B,512,20) -> relu -> (B,64,18)
                # Output positions are packed in pairs: even w on PSUM/SBUF
                # partitions 0-63, odd w on 64-127. The two M=64 accumulation
                # groups land on different PE column groups and execute
                # concurrently (~2x conv1 throughput).
                # Both halves open their accumulation group up front (their
                # clears land before any drain writes), then accumulate.
                # The group checker doesn't model a two-col-group interleave
                # in one bank, hence skip_group_check.
                h1 = sb.tile([128, 9 * bt], dt.bfloat16, tag="h1")
                for u in range(9):
                    p1 = ps.tile([128, bt], dt.float32, tag="c1")
                    for q in range(4):
                        for k in range(3):
                            nc.tensor.matmul(
                                p1[0:64, :],
                                w1_sb[q][:, k * 64 : (k + 1) * 64],
                                x_sb[q][:, (2 * u + k) * bt : (2 * u + k + 1) * bt],
                                start=(q == 0 and k == 0),
                                stop=(q == 3 and k == 2),
                                skip_group_check=True,
                            )
                            nc.tensor.matmul(
                                p1[64:128, :],
                                w1_sb[q][:, k * 64 : (k + 1) * 64],
                                x_sb[q][:, (2 * u + 1 + k) * bt : (2 * u + 2 + k) * bt],
                                start=(q == 0 and k == 0),
                                stop=(q == 3 and k == 2),
                                skip_group_check=True,
                            )
                    nc.scalar.activation(h1[:, u * bt : (u + 1) * bt], p1[:], AF.Relu)

                # conv2: -> relu -> (B,128,14)
                # h1's parity-split layout lets adjacent taps fuse into one
                # full 128-row contraction (tap k on rows 0-63, tap k+1 on
                # 64-127), with zero-padded weight blocks at the edges so
                # every matmul is full-height: 3 matmuls per position
                # instead of 5. Host-prepped blocks (see _prep_inputs):
                #   even w': [k0;k1] [k2;k3] [k4;0 ]  at h1 cols t', t'+1, t'+2
                #   odd  w': [0;k0] [k1;k2] [k3;k4]   at h1 cols t', t'+1, t'+2
                h2 = sb.tile([128, 14 * bt], dt.bfloat16, tag="h2")
                for w in range(14):
                    t0 = w // 2
                    blk0 = 0 if w % 2 == 0 else 3
                    p2 = ps.tile([128, bt], dt.float32, tag="c2")
                    for j in range(3):
                        blk = blk0 + j
                        nc.tensor.matmul(
                            p2[:],
                            w2_sb[:, blk * 128 : (blk + 1) * 128],
                            h1[:, (t0 + j) * bt : (t0 + j + 1) * bt],
                            start=(j == 0),
                            stop=(j == 2),
                        )
                    nc.vector.tensor_relu(h2[:, w * bt : (w + 1) * bt], p2[:])

                # conv3: -> relu -> (B,256,8) as two 128-channel tiles
                h3 = [
                    sb.tile([128, 8 * bt], dt.bfloat16, tag=f"h3_{m}", name=f"h3_{m}") for m in range(2)
                ]
                for w in range(8):
                    for m in range(2):
                        p3 = ps.tile([128, bt], dt.float32, tag="c3")
                        for k in range(7):
                            nc.tensor.matmul(
                                p3[:],
                                w3_sb[:, k * 256 + m * 128 : k * 256 + (m + 1) * 128],
                                h2[:, (w + k) * bt : (w + k + 1) * bt],
                                start=(k == 0),
                                stop=(k == 6),
                            )
                        nc.vector.tensor_relu(h3[m][:, w * bt : (w + 1) * bt], p3[:])

                # maxpool k=2 s=2: (B,256,8) -> (B,256,4)
                pooled = [
                    sb.tile([128, 4 * bt], dt.bfloat16, tag=f"pool_{m}", name=f"pool_{m}")
                    for m in range(2)
                ]
                for m in range(2):
                    for p in range(4):
                        nc.vector.tensor_max(
                            pooled[m][:, p * bt : (p + 1) * bt],
                            h3[m][:, (2 * p) * bt : (2 * p + 1) * bt],
                            h3[m][:, (2 * p + 1) * bt : (2 * p + 2) * bt],
                        )

                # mlp1: (B,1024)->(B,256), f = c*4 + wp
                g1 = [sb.tile([128, bt], dt.bfloat16, tag=f"g1_{j}", name=f"g1_{j}") for j in range(2)]
                for j in range(2):
                    pm = ps.tile([128, bt], dt.float32, tag="m")
                    for wp in range(4):
                        for q in range(2):
                            nc.tensor.matmul(
                                pm[:],
                                m1_sb[wp][q][:, j * 128 : (j + 1) * 128],
                                pooled[q][:, wp * bt : (wp + 1) * bt],
                                start=(wp == 0 and q == 0),
                                stop=(wp == 3 and q == 1),
                            )
                    nc.vector.tensor_relu(g1[j][:], pm[:])

                # mlp2: (B,256)->(B,128)
                g2 = sb.tile([128, bt], dt.bfloat16, tag="g2")
                pm = ps.tile([128, bt], dt.float32, tag="m")
                for q in range(2):
                    nc.tensor.matmul(
                        pm[:], m2_sb[q][:], g1[q][:], start=(q == 0), stop=(q == 1)
                    )
                nc.vector.tensor_relu(g2[:], pm[:])

                # mlp3: (B,128)->(B,1)
                pm = ps.tile([1, bt], dt.float32, tag="m")
                nc.tensor.matmul(pm[:], m3_sb[:], g2[:], start=True, stop=True)
                y_sb = sb.tile([1, max(TILES)], dt.float32, tag="y_sb", bufs=2)
                nc.vector.tensor_copy(y_sb[:, :bt], pm[:])
                nc.sync.dma_start(y_d[:, boff : boff + bt], y_sb[:, :bt])

                boff += bt

    nc.compile()
    return nc


def _prep_inputs(x, kernel_1, kernel_2, kernel_3, mlp_weight_1, mlp_weight_2, mlp_weight_3):
    """Host-side sharding + layout prep. Returns in_maps (one dict per core)."""
    w1 = np.ascontiguousarray(
        kernel_1.transpose(1, 2, 0).reshape(512, 3 * 64)
    ).astype(BF16)
    # conv2 tap-pair blocks for the parity-split h1 layout: column block j is
    # a (128, 128) lhsT whose rows 0-63 multiply h1's even half and rows
    # 64-127 the odd half. Blocks 0-2 serve even output positions
    # ([k0;k1] [k2;k3] [k4;0]), blocks 3-5 odd ones ([0;k0] [k1;k2] [k3;k4]).
    k2t = kernel_2.transpose(1, 2, 0).astype(np.float32)  # (64, 5, 128)
    z = np.zeros((64, 128), np.float32)
    blocks = [
        np.concatenate([k2t[:, 0], k2t[:, 1]], axis=0),
        np.concatenate([k2t[:, 2], k2t[:, 3]], axis=0),
        np.concatenate([k2t[:, 4], z], axis=0),
        np.concatenate([z, k2t[:, 0]], axis=0),
        np.concatenate([k2t[:, 1], k2t[:, 2]], axis=0),
        np.concatenate([k2t[:, 3], k2t[:, 4]], axis=0),
    ]
    w2 = np.ascontiguousarray(np.concatenate(blocks, axis=1)).astype(BF16)
    w3 = np.ascontiguousarray(
        kernel_3.transpose(1, 2, 0).reshape(128, 7 * 256)
    ).astype(BF16)
    # W1 row f = c*4 + wp  ->  m1 row = wp*256 + c
    m1 = np.ascontiguousarray(
        mlp_weight_1.reshape(256, 4, 256).transpose(1, 0, 2).reshape(1024, 256)
    ).astype(BF16)
    m2 = mlp_weight_2.astype(BF16)
    m3 = mlp_weight_3.astype(BF16)

    xb = x.astype(BF16)
    in_maps = []
    for c in range(N_CORES):
        xc = np.ascontiguousarray(
            xb[c * BC : (c + 1) * BC].transpose(1, 2, 0)
        )  # (512, 20, BC)
        in_maps.append(
            {"x": xc, "w1": w1, "w2": w2, "w3": w3, "m1": m1, "m2": m2, "m3": m3}
        )
    return in_maps


def run(inputs, trace=False, **kw):
    """Compile (cached), run on 8 cores, return (y_full, BassKernelResults)."""
    from concourse import bass_utils

    if "nc" not in _compiled:
        _compiled["nc"] = _build()
    nc = _compiled["nc"]
    in_maps = _prep_inputs(**inputs)
    res = bass_utils.run_bass_kernel_spmd(
        nc, in_maps, core_ids=list(range(N_CORES)), trace=trace, **kw
    )
    y = np.concatenate(
        [res.results[c]["y"].reshape(BC, 1) for c in range(N_CORES)], axis=0
    )
    return y.astype(np.float32), res


def kernel(**inputs):
    inputs = {k: np.asarray(v) for k, v in inputs.items()}
    y, _ = run(inputs)
    return y


if __name__ == "__main__":
    rng = np.random.default_rng(0)
    inputs = {
        "x": rng.standard_normal((B, E, W), dtype=np.float32),
        "kernel_1": rng.standard_normal((64, 512, 3), dtype=np.float32),
        "kernel_2": rng.standard_normal((128, 64, 5), dtype=np.float32),
        "kernel_3": rng.standard_normal((256, 128, 7), dtype=np.float32),
        "mlp_weight_1": rng.standard_normal((1024, 256), dtype=np.float32),
        "mlp_weight_2": rng.standard_normal((256, 128), dtype=np.float32),
        "mlp_weight_3": rng.standard_normal((128, 1), dtype=np.float32),
    }
    y = kernel(**inputs)
    print("out", y.shape, y.dtype, y[:4, 0])


# revision 20
# speedup vs baseline: 3.4666x; 1.1231x over previous
"""Trainium2 Bass kernel for nn_CNNFromScratch (dense 1-D CNN + MLP head).

Strategy
--------
Pure data parallelism: the batch axis (8192) is split across 8 NeuronCores
(1024 samples each); conv kernels and MLP weights are replicated.

Per core, everything is expressed as TensorE matmuls with the contraction
(input channels x taps) on the partition axis:

  - x is pre-transposed on host to (C=512, B, W=20) and cast to bf16, so a
    c-chunk tile loads as (128 partitions, bt*20) with perfectly contiguous
    per-partition DMA runs.
  - conv_k == sum over taps of  W_tap^T @ x[:, :, w+tap]  accumulated in PSUM.
  - Activations stay on-chip (SBUF, bf16) between layers; layout is
    (C_out partitions, w-major * batch free), which feeds the next conv's
    matmuls with plain contiguous slices.
  - maxpool = DVE tensor_max of two strided slices; MLP = accumulated
    matmuls over (channel, pooled-position) chunks.

Matmul inputs are bf16 (1 cycle/row on PE), accumulation is fp32 in PSUM.
"""

import sys

sys.path.insert(0, "/opt/trn_rl_repo")

import numpy as np
import ml_dtypes

N_CORES = 8
B, E, W = 8192, 512, 20
BC = B // N_CORES  # samples per core
# bt=512 keeps per-matmul stream time above the LDWEIGHTS shadow; tile 0's
# conv1 runs chunk-outer so the PE starts as soon as the first c-chunk lands.
TILES = [512, 512]
assert sum(TILES) == BC

BF16 = ml_dtypes.bfloat16

_compiled = {}


def _build():
    import concourse.bass as bass
    from concourse import bacc, mybir
    import concourse.tile as tile

    dt = mybir.dt
    AF = mybir.ActivationFunctionType

    nc = bacc.Bacc(
        "TRN2",
        target_bir_lowering=False,
        debug=False,
        enable_asserts=False,
        num_devices=N_CORES,
    )

    x_d = nc.dram_tensor("x", (E, W, BC), dt.bfloat16, kind="ExternalInput").ap()
    w1_d = nc.dram_tensor("w1", (512, 3 * 64), dt.bfloat16, kind="ExternalInput").ap()
    w2_d = nc.dram_tensor("w2", (128, 6 * 128), dt.bfloat16, kind="ExternalInput").ap()
    w3_d = nc.dram_tensor("w3", (128, 7 * 256), dt.bfloat16, kind="ExternalInput").ap()
    m1_d = nc.dram_tensor("m1", (1024, 256), dt.bfloat16, kind="ExternalInput").ap()
    m2_d = nc.dram_tensor("m2", (256, 128), dt.bfloat16, kind="ExternalInput").ap()
    m3_d = nc.dram_tensor("m3", (128, 1), dt.bfloat16, kind="ExternalInput").ap()
    y_d = nc.dram_tensor("y", (1, BC), dt.float32, kind="ExternalOutput").ap()

    with tile.TileContext(nc) as tc:
        with (
            tc.tile_pool(name="sb", bufs=1) as sb,
            tc.tile_pool(name="ps", bufs=8, space="PSUM") as ps,
        ):
            # ---- weights (resident for the whole kernel) ----
            # conv1 weights + tile-0 x are emitted first so the PE can start
            # ~8us in; the remaining weights load under tile-0's conv1.
            w1_sb = []
            for q in range(4):
                t = sb.tile([128, 3 * 64], dt.bfloat16, tag=f"w1_{q}")
                nc.sync.dma_start(t[:], w1_d[q * 128 : (q + 1) * 128, :])
                w1_sb.append(t)

            def load_x(bt, boff):
                x_sb = []
                for q in range(4):
                    t = sb.tile(
                        [128, max(TILES) * 20],
                        dt.bfloat16,
                        tag="x",
                        bufs=5,
                        name=f"x_{q}",
                    )
                    t3 = t[:, : bt * 20].rearrange("p (w b) -> p w b", b=bt)
                    nc.sync.dma_start(
                        t3, x_d[q * 128 : (q + 1) * 128, :, boff : boff + bt]
                    )
                    x_sb.append(t[:, : bt * 20])
                return x_sb

            x_first = load_x(TILES[0], 0)

            w2_sb = sb.tile([128, 6 * 128], dt.bfloat16, tag="w2")
            nc.sync.dma_start(w2_sb[:], w2_d[:, :])
            w3_sb = sb.tile([128, 7 * 256], dt.bfloat16, tag="w3")
            nc.sync.dma_start(w3_sb[:], w3_d[:, :])
            m1_sb = []
            for wp in range(4):
                row = []
                for q in range(2):
                    t = sb.tile([128, 256], dt.bfloat16, tag=f"m1_{wp}_{q}")
                    r0 = wp * 256 + q * 128
                    nc.sync.dma_start(t[:], m1_d[r0 : r0 + 128, :])
                    row.append(t)
                m1_sb.append(row)
            m2_sb = []
            for q in range(2):
                t = sb.tile([128, 128], dt.bfloat16, tag=f"m2_{q}")
                nc.sync.dma_start(t[:], m2_d[q * 128 : (q + 1) * 128, :])
                m2_sb.append(t)
            m3_sb = sb.tile([128, 1], dt.bfloat16, tag="m3")
            nc.sync.dma_start(m3_sb[:], m3_d[:, :])

            # ---- per-batch-tile pipeline ----
            boff = 0
            for ti, bt in enumerate(TILES):
                # x: 4 c-chunk tiles, (128, 20, bt); tag-shared slots sized to
                # the largest bt so later tiles prefetch into freed slots.
                # SBUF layout is w-major (free = w*bt + b) so conv1's matmul
                # rhs slices are contiguous — strided moving operands stream
                # at half rate and keep the PE clock-gate cold.
                x_sb = x_first if ti == 0 else load_x(bt, boff)

                # conv1: (B,512,20) -> relu -> (B,64,18)
                # Output positions are packed in pairs: even w on PSUM/SBUF
                # partitions 0-63, odd w on 64-127. The two M=64 accumulation
                # groups land on different PE column groups and execute
                # concurrently (~2x conv1 throughput).
                # Both halves open their accumulation group up front (their
                # clears land before any drain writes), then accumulate.
                # The group checker doesn't model a two-col-group interleave
                # in one bank, hence skip_group_check.
                h1 = sb.tile([128, 9 * bt], dt.bfloat16, tag="h1")

                def conv1_mms(p1, u, q):
                    for k in range(3):
                        nc.tensor.matmul(
                            p1[0:64, :],
                            w1_sb[q][:, k * 64 : (k + 1) * 64],
                            x_sb[q][:, (2 * u + k) * bt : (2 * u + k + 1) * bt],
                            start=(q == 0 and k == 0),
                            stop=(q == 3 and k == 2),
                            skip_group_check=True,
                        )
                        nc.tensor.matmul(
                            p1[64:128, :],
                            w1_sb[q][:, k * 64 : (k + 1) * 64],
                            x_sb[q][:, (2 * u + 1 + k) * bt : (2 * u + 2 + k) * bt],
                            start=(q == 0 and k == 0),
                            stop=(q == 3 and k == 2),
                            skip_group_check=True,
                        )

                if ti == 0:
                    # Chunk-outer: all matmuls for chunk q across a block of
                    # output pairs before moving to chunk q+1, so the PE
                    # starts when the first c-chunk DMA lands instead of
                    # waiting for all four.
                    for u0, u1 in ((0, 8), (8, 9)):
                        p1s = [
                            ps.tile([128, bt], dt.float32, tag="ps", name=f"p1_{u}")
                            for u in range(u0, u1)
                        ]
                        for q in range(4):
                            for u in range(u0, u1):
                                conv1_mms(p1s[u - u0], u, q)
                        for u in range(u0, u1):
                            nc.scalar.activation(
                                h1[:, u * bt : (u + 1) * bt], p1s[u - u0][:], AF.Relu
                            )
                else:
                    for u in range(9):
                        p1 = ps.tile([128, bt], dt.float32, tag="ps")
                        for q in range(4):
                            conv1_mms(p1, u, q)
                        nc.scalar.activation(
                            h1[:, u * bt : (u + 1) * bt], p1[:], AF.Relu
                        )

                # conv2: -> relu -> (B,128,14)
                # h1's parity-split layout lets adjacent taps fuse into one
                # full 128-row contraction (tap k on rows 0-63, tap k+1 on
                # 64-127), with zero-padded weight blocks at the edges so
                # every matmul is full-height: 3 matmuls per position
                # instead of 5. Host-prepped blocks (see _prep_inputs):
                #   even w': [k0;k1] [k2;k3] [k4;0 ]  at h1 cols t', t'+1, t'+2
                #   odd  w': [0;k0] [k1;k2] [k3;k4]   at h1 cols t', t'+1, t'+2
                h2 = sb.tile([128, 14 * bt], dt.bfloat16, tag="h2")
                for w in range(14):
                    t0 = w // 2
                    blk0 = 0 if w % 2 == 0 else 3
                    p2 = ps.tile([128, bt], dt.float32, tag="ps")
                    for j in range(3):
                        blk = blk0 + j
                        nc.tensor.matmul(
                            p2[:],
                            w2_sb[:, blk * 128 : (blk + 1) * 128],
                            h1[:, (t0 + j) * bt : (t0 + j + 1) * bt],
                            start=(j == 0),
                            stop=(j == 2),
                        )
                    nc.vector.tensor_relu(h2[:, w * bt : (w + 1) * bt], p2[:])

                # conv3: -> relu -> (B,256,8) as two 128-channel tiles
                h3 = [
                    sb.tile([128, 8 * bt], dt.bfloat16, tag=f"h3_{m}", name=f"h3_{m}") for m in range(2)
                ]
                for w in range(8):
                    for m in range(2):
                        p3 = ps.tile([128, bt], dt.float32, tag="ps")
                        for k in range(7):
                            nc.tensor.matmul(
                                p3[:],
                                w3_sb[:, k * 256 + m * 128 : k * 256 + (m + 1) * 128],
                                h2[:, (w + k) * bt : (w + k + 1) * bt],
                                start=(k == 0),
                                stop=(k == 6),
                            )
                        nc.vector.tensor_relu(h3[m][:, w * bt : (w + 1) * bt], p3[:])

                # maxpool k=2 s=2: (B,256,8) -> (B,256,4)
                pooled = [
                    sb.tile([128, 4 * bt], dt.bfloat16, tag=f"pool_{m}", name=f"pool_{m}")
                    for m in range(2)
                ]
                for m in range(2):
                    for p in range(4):
                        nc.vector.tensor_max(
                            pooled[m][:, p * bt : (p + 1) * bt],
                            h3[m][:, (2 * p) * bt : (2 * p + 1) * bt],
                            h3[m][:, (2 * p + 1) * bt : (2 * p + 2) * bt],
                        )

                # mlp1: (B,1024)->(B,256), f = c*4 + wp
                g1 = [sb.tile([128, bt], dt.bfloat16, tag=f"g1_{j}", name=f"g1_{j}") for j in range(2)]
                for j in range(2):
                    pm = ps.tile([128, bt], dt.float32, tag="ps")
                    for wp in range(4):
                        for q in range(2):
                            nc.tensor.matmul(
                                pm[:],
                                m1_sb[wp][q][:, j * 128 : (j + 1) * 128],
                                pooled[q][:, wp * bt : (wp + 1) * bt],
                                start=(wp == 0 and q == 0),
                                stop=(wp == 3 and q == 1),
                            )
                    nc.vector.tensor_relu(g1[j][:], pm[:])

                # mlp2: (B,256)->(B,128)
                g2 = sb.tile([128, bt], dt.bfloat16, tag="g2")
                pm = ps.tile([128, bt], dt.float32, tag="ps")
                for q in range(2):
                    nc.tensor.matmul(
                        pm[:], m2_sb[q][:], g1[q][:], start=(q == 0), stop=(q == 1)
                    )
                nc.vector.tensor_relu(g2[:], pm[:])

                # mlp3: (B,128)->(B,1)
                pm = ps.tile([1, bt], dt.float32, tag="ps")
                nc.tensor.matmul(pm[:], m3_sb[:], g2[:], start=True, stop=True)
                y_sb = sb.tile([1, max(TILES)], dt.float32, tag="y_sb", bufs=2)
                nc.vector.tensor_copy(y_sb[:, :bt], pm[:])
                nc.sync.dma_start(y_d[:, boff : boff + bt], y_sb[:, :bt])

                boff += bt

    nc.compile()
    return nc


def _prep_inputs(x, kernel_1, kernel_2, kernel_3, mlp_weight_1, mlp_weight_2, mlp_weight_3):
    """Host-side sharding + layout prep. Returns in_maps (one dict per core)."""
    w1 = np.ascontiguousarray(
        kernel_1.transpose(1, 2, 0).reshape(512, 3 * 64)
    ).astype(BF16)
    # conv2 tap-pair blocks for the parity-split h1 layout: column block j is
    # a (128, 128) lhsT whose rows 0-63 multiply h1's even half and rows
    # 64-127 the odd half. Blocks 0-2 serve even output positions
    # ([k0;k1] [k2;k3] [k4;0]), blocks 3-5 odd ones ([0;k0] [k1;k2] [k3;k4]).
    k2t = kernel_2.transpose(1, 2, 0).astype(np.float32)  # (64, 5, 128)
    z = np.zeros((64, 128), np.float32)
    blocks = [
        np.concatenate([k2t[:, 0], k2t[:, 1]], axis=0),
        np.concatenate([k2t[:, 2], k2t[:, 3]], axis=0),
        np.concatenate([k2t[:, 4], z], axis=0),
        np.concatenate([z, k2t[:, 0]], axis=0),
        np.concatenate([k2t[:, 1], k2t[:, 2]], axis=0),
        np.concatenate([k2t[:, 3], k2t[:, 4]], axis=0),
    ]
    w2 = np.ascontiguousarray(np.concatenate(blocks, axis=1)).astype(BF16)
    w3 = np.ascontiguousarray(
        kernel_3.transpose(1, 2, 0).reshape(128, 7 * 256)
    ).astype(BF16)
    # W1 row f = c*4 + wp  ->  m1 row = wp*256 + c
    m1 = np.ascontiguousarray(
        mlp_weight_1.reshape(256, 4, 256).transpose(1, 0, 2).reshape(1024, 256)
    ).astype(BF16)
    m2 = mlp_weight_2.astype(BF16)
    m3 = mlp_weight_3.astype(BF16)

    xb = x.astype(BF16)
    in_maps = []
    for c in range(N_CORES):
        xc = np.ascontiguousarray(
            xb[c * BC : (c + 1) * BC].transpose(1, 2, 0)
        )  # (512, 20, BC)
        in_maps.append(
            {"x": xc, "w1": w1, "w2": w2, "w3": w3, "m1": m1, "m2": m2, "m3": m3}
        )
    return in_maps


def run(inputs, trace=False, **kw):
    """Compile (cached), run on 8 cores, return (y_full, BassKernelResults)."""
    from concourse import bass_utils

    if "nc" not in _compiled:
        _compiled["nc"] = _build()
    nc = _compiled["nc"]
    in_maps = _prep_inputs(**inputs)
    res = bass_utils.run_bass_kernel_spmd(
        nc, in_maps, core_ids=list(range(N_CORES)), trace=trace, **kw
    )
    y = np.concatenate(
        [res.results[c]["y"].reshape(BC, 1) for c in range(N_CORES)], axis=0
    )
    return y.astype(np.float32), res


def kernel(**inputs):
    inputs = {k: np.asarray(v) for k, v in inputs.items()}
    y, _ = run(inputs)
    return y


if __name__ == "__main__":
    rng = np.random.default_rng(0)
    inputs = {
        "x": rng.standard_normal((B, E, W), dtype=np.float32),
        "kernel_1": rng.standard_normal((64, 512, 3), dtype=np.float32),
        "kernel_2": rng.standard_normal((128, 64, 5), dtype=np.float32),
        "kernel_3": rng.standard_normal((256, 128, 7), dtype=np.float32),
        "mlp_weight_1": rng.standard_normal((1024, 256), dtype=np.float32),
        "mlp_weight_2": rng.standard_normal((256, 128), dtype=np.float32),
        "mlp_weight_3": rng.standard_normal((128, 1), dtype=np.float32),
    }
    y = kernel(**inputs)
    print("out", y.shape, y.dtype, y[:4, 0])
